# revision 28
# baseline (speedup 1.0000x reference)
"""Bark-style causal self-attention on 8 Trainium2 NeuronCores.

Problem (hardcoded): B=4, S=1024, D=1024, H=16, hd=64, fp32 I/O.

Sharding: 8 cores = 4 batches x 2 head-groups (8 heads each).

Single fully-woven pipeline per core, designed to keep the PE tensor engine
continuously busy (the cost model's p-state ramp halves the PE clock after
any idle gap):

  - qk^T projection in 4 k-major waves of 4 PSUM tiles; score matmuls for
    pair w-1 are woven between wave w's projection matmuls, so the Act
    engine's exp marathon (the only engine that can run Exp) hides under
    projection compute.
  - scores are computed transposed per head pair (tile_position row
    packing), exp'd on Act (both heads per instruction), causal-masked on
    DVE.
  - PV runs in NATURAL orientation: stationary = p^T 128x128 blocks,
    moving = V with an appended ones column (65 cols) -> ctx[q, d] comes
    out with the softmax denominator as a per-partition scalar column.
    This halves PV PE-cycles vs streaming queries and makes normalization
    a cheap reciprocal + tensor_scalar_mul per 128-query block.
  - normalized ctx blocks are transposed back with PE identity-matmuls
    (1 cycle/row) into ctx^T for the output projection.
  - out^T partial = w_out.T @ ctx^T accumulated pair 0..2 then pair 3
    (woven with pair 3's transposes) in 4 waves of 4 PSUM tiles; the two
    cores of a batch hold partial sums which are combined on the host.
"""

from contextlib import ExitStack

import numpy as np
import ml_dtypes

import concourse.bass as bass
import concourse.tile as tile
import concourse.mybir as mybir
from concourse.bass_utils import run_bass_kernel_spmd
from concourse.vector_clock import ScopedClock


# --------------------------------------------------------------------------
# Workaround for the walrus build in this container, which accepts at most
# ONE sync-wait command per instruction (two on EventSemaphore).  Stock Tile
# emits instructions with several waits; we legalize the program after
# TileContext exit.
# --------------------------------------------------------------------------

def _patched_drain_and_barrier(self, tick_clock, wait_clock):
    drain_inst = self.nc.sync.drain()
    wait_clock.add_sem_waits(
        drain_inst.ins, ScopedClock({None: tick_clock.global_clock})
    )
    si = drain_inst.ins.sync_info
    waits = list(si.on_wait or []) if si is not None else []
    if len(waits) > 1:
        si.on_wait = [waits[0]]
        for w in waits[1:]:
            extra = self.nc.sync.drain()
            esi = extra.ins.sync_info
            if esi is None:
                extra.ins.sync_info = mybir.SyncInfo(on_wait=[w], on_update=[])
            else:
                esi.on_wait = [w]

    self.nc.all_engine_barrier()
    assert self.sems is not None
    popped = self.nc._tile_sem_poison_stack.pop()
    assert popped is self._sem_poison
    self.nc.clear_and_free_semaphores(list(self.sems.allocated().values()))
    self.nc.all_engine_barrier()


tile.TileContext._drain_and_barrier = _patched_drain_and_barrier


def _legalize_waits_json(raw: bytes) -> bytes:
    """Split multi-wait instructions by inserting single-wait NoOp carriers
    immediately before them on the same engine (pure in-stream split: all
    waits still execute before the instruction, in the same order)."""
    import orjson

    j = orjson.loads(raw)
    for f in j["functions"]:
        for b in f["blocks"]:
            out = []
            for inst in b["instructions"]:
                si = inst.get("sync_info") or {}
                waits = si.get("on_wait") or []
                cap = 2 if inst.get("opcode") == "EventSemaphore" else 1
                if len(waits) > cap:
                    excess, keep = waits[:-cap], waits[-cap:]
                    for k, w in enumerate(excess):
                        out.append({
                            "debug": inst.get("debug", 0),
                            "engine": inst["engine"],
                            "ins": [],
                            "name": f"{inst['name']}-lw{k}",
                            "opcode": "NoOp",
                            "outs": [],
                            "sync_info": {"on_wait": [w]},
                        })
                    si["on_wait"] = keep
                    inst["sync_info"] = si
                out.append(inst)
            b["instructions"] = out
    return orjson.dumps(j)


BF16 = mybir.dt.bfloat16
F32 = mybir.dt.float32
NPBF16 = ml_dtypes.bfloat16

B, S, D, H, HD = 4, 1024, 1024, 16, 64
NCORES = 8
HPC = 8          # heads per core
PAIRS = 4        # head pairs per core
KCH = 8          # 128-row chunks of the D contraction
SCALE = 1.0 / np.sqrt(HD)

# Set by test harness to capture a profile; read back from LAST_RESULTS.
TRACE = False
LAST_RESULTS = None

_CACHE = {}


def _chunks(kb):
    """Column chunks for key-block kb: causal cols [kb*128, S) split at the
    absolute 512 boundary (PSUM bank / q-half boundary)."""
    lo = kb * 128
    if lo < 512:
        return [(lo, 512), (512, 1024)]
    return [(lo, 1024)]


def _emit(tc, io, ctx):
    nc = tc.nc
    hsT, wqk, qkb, wv, wout, outb, tri, eye, outT = (
        io["hsT"], io["wqk"], io["qkb"], io["wv"], io["wout"], io["outb"],
        io["tri"], io["eye"], io["outT"],
    )
    Exp = mybir.ActivationFunctionType.Exp
    Ident = mybir.ActivationFunctionType.Identity

    persist = ctx.enter_context(tc.tile_pool(name="persist", bufs=1))

    def load(name, src, shape, dtype=BF16):
        t = persist.tile(shape, dtype, name=name, tag=name)
        nc.sync.dma_start(out=t[:, :], in_=src)
        return t

    # Warmup source for dummy matmuls (Pool memset, no input deps, runs at
    # t~0).  The dummies keep the PE p-state ramp alive through the
    # load-supply-bound first wave: any PE idle gap halves the modeled PE
    # clock for the next 3us.
    dmsrc = persist.tile([128, 512], BF16, name="dmsrc", tag="dmsrc")
    nc.gpsimd.memset(dmsrc[:, 0:128], 0.0)
    nc.gpsimd.memset(dmsrc[:, 128:512], 0.0)
    dmrd = persist.tile([128, 2], F32, name="dmrd", tag="dmrd")

    # ---- resident SBUF tensors (loads in consumption order) --------------
    # Full-chunk loads: the 625ns HWDGE generation cost per DMA means small
    # slices make supply HWDGE-bound; [128,1024]bf16 keeps it transfer-bound.
    wqk_sb, hsT_sb, wv_sb = [], [], []
    for k in range(KCH):
        r0, r1 = k * 128, (k + 1) * 128
        ht = persist.tile([128, S], BF16, name=f"hsT{k}", tag=f"hsT{k}")
        vt = persist.tile([128, 512], BF16, name=f"wv{k}", tag=f"wv{k}")
        if k == 0:
            nc.sync.dma_start(out=ht[:, 0:512], in_=hsT[r0:r1, 0:512])
            nc.sync.dma_start(out=vt[:, :], in_=wv[r0:r1, :])
            nc.sync.dma_start(out=ht[:, 512:1024], in_=hsT[r0:r1, 512:1024])
        else:
            nc.sync.dma_start(out=vt[:, :], in_=wv[r0:r1, :])
            nc.sync.dma_start(out=ht[:, :], in_=hsT[r0:r1, :])
        hsT_sb.append(ht)
        wv_sb.append(vt)
    for k in range(KCH):
        r0, r1 = k * 128, (k + 1) * 128
        wqk_sb.append(load(f"wqk{k}", wqk[r0:r1, :], [128, 1024]))
    qkb_sb = load("qkb", qkb[:, :], [128, 8], F32)
    wout_sb = [load(f"wout{p}", wout[p * 128:(p + 1) * 128, :], [128, 1024])
               for p in range(PAIRS)]
    outb_sb = load("outb", outb[:, :], [128, 8], F32)
    tri_sb = load("tri", tri[:, :], [128, 128])
    eye_sb = load("eye", eye[:, :], [128, 128])

    # projection outputs
    qkT_sb = [persist.tile([128, S], BF16, name=f"qkT{m}", tag=f"qkT{m}")
              for m in range(8)]   # 0-3: q pairs, 4-7: k pairs
    v_sb = [persist.tile([128, HPC * 65], BF16, name=f"v{s}", tag=f"v{s}")
            for s in range(8)]
    ctxT_sb = [persist.tile([128, S], BF16, name=f"ctxT{p}", tag=f"ctxT{p}")
               for p in range(PAIRS)]

    # rotating SBUF pools
    pt_pool = ctx.enter_context(tc.tile_pool(name="pt", bufs=1))
    cnat_pool = ctx.enter_context(tc.tile_pool(name="cnat", bufs=2))
    rec_pool = ctx.enter_context(tc.tile_pool(name="rec", bufs=8))
    osb_pool = ctx.enter_context(tc.tile_pool(name="osb", bufs=8))
    sums_pool = ctx.enter_context(tc.tile_pool(name="sums", bufs=4))

    # sT psum pool opened first (outlives the sequential proj/ctx/ops pools)
    sT_pool = ctx.enter_context(tc.tile_pool(name="sTp", bufs=2,
                                             space="PSUM"))

    pt_t = {}    # (p, kb, ci) -> (tile, c0, width)

    def emit_chunk(p, kb, ci, c0, c1):
        """Score matmuls (PE) + exp (Act) + causal mask (DVE) for chunk
        (kb, ci) of pair p, covering absolute cols [c0, c1)."""
        wc = c1 - c0
        sT = sT_pool.tile([128, 2, 512], F32, name=f"sT{p}_{kb}_{ci}",
                          tag="sT")
        for t in range(2):
            nc.tensor.matmul(
                sT[:, t, 0:wc],
                lhsT=qkT_sb[4 + p][64 * t:64 * t + 64,
                                   kb * 128:(kb + 1) * 128],
                rhs=qkT_sb[p][64 * t:64 * t + 64, c0:c1],
                start=True, stop=True,
                tile_position=(64 * t, 0))
        pt = pt_pool.tile([128, 2, wc], BF16, name=f"pt{p}_{kb}_{ci}",
                          tag=f"pt{wc}", bufs=_PT_BUFS[wc])
        nc.scalar.activation(pt[:, :, 0:wc], sT[:, :, 0:wc], Exp, scale=SCALE)
        if c0 == kb * 128:
            pm = pt[:, :, 0:128]
            tri3 = tri_sb.rearrange("p (o c) -> p o c", o=1)
            tri_b, _ = bass.broadcast_tensor_aps(tri3, pm)
            nc.vector.tensor_mul(pm, pm, tri_b)
        pt_t[(p, kb, ci)] = (pt, c0, wc)

    def score_sched(p):
        """List of chunk-emit thunks for pair p (12 chunks, kb-major)."""
        out = []
        for kb in range(KCH):
            for ci, (c0, c1) in enumerate(_chunks(kb)):
                out.append((p, kb, ci, c0, c1))
        return out

    # ---- phase 0: V projection, k-major (supply-friendly: consumes only
    # hsT+wv loads while the wqk stream still arrives) ------------------------
    dm = [sT_pool.tile([128, 2, 512], F32, name=f"dm{i}", tag="sT")
          for i in range(1)]

    def dummy_mm(i, cols=512):
        nc.tensor.matmul(dm[i][:, 0, 0:cols], lhsT=dmsrc[:, 0:128],
                         rhs=dmsrc[:, 0:cols], start=True, stop=True)

    for _ in range(3):
        dummy_mm(0, cols=128)
    for _ in range(4):
        dummy_mm(0)

    v_cm = tc.tile_pool(name="vp", bufs=4, space="PSUM")
    v_pool = v_cm.__enter__()
    for gi, group in enumerate((range(0, 4), range(4, 8))):
        vps = {}
        for k in range(KCH):
            for si in group:
                if k == 0:
                    vps[si] = v_pool.tile([128, 512], F32, name=f"vps{si}",
                                          tag="vp")
                nc.tensor.matmul(
                    vps[si][:, :],
                    lhsT=hsT_sb[k][:, si * 128:(si + 1) * 128],
                    rhs=wv_sb[k][:, :],
                    start=(k == 0), stop=(k == KCH - 1))
                if k == KCH - 1:
                    v3 = v_sb[si].rearrange("p (h c) -> p h c", c=65)
                    nc.vector.tensor_copy(
                        v3[:, :, 0:64],
                        vps[si].rearrange("p (h c) -> p h c", c=64))
                    nc.gpsimd.memset(v3[:, :, 64:65], 1.0)
            if gi == 0 and 1 <= k <= 7:
                # first group is load-supply-bound; pad the PE
                dummy_mm(0)
    # bridge the v->qk pool handover
    nc.vector.tensor_copy(dmrd[:, 0:1], dm[0][:, 0, 0:1])
    for _ in range(3):
        dummy_mm(0)
    v_cm.__exit__(None, None, None)

    # ---- phase 1: qk^T projection, 4 k-major waves + woven scores --------
    proj_cm = tc.tile_pool(name="pj", bufs=4, space="PSUM")
    proj_pool = proj_cm.__enter__()

    for w in range(4):
        tiles = [(m, n) for m in (w, 4 + w) for n in range(2)]
        ps = {}
        sched = score_sched(w - 1) if w >= 1 else []
        ci = 0
        for k in range(KCH):
            for (m, n) in tiles:
                if k == 0:
                    ps[(m, n)] = proj_pool.tile([128, 512], F32,
                                                name=f"pj{w}_{m}_{n}",
                                                tag="pj")
                nc.tensor.matmul(
                    ps[(m, n)][:, :],
                    lhsT=wqk_sb[k][:, m * 128:(m + 1) * 128],
                    rhs=hsT_sb[k][:, n * 512:(n + 1) * 512],
                    start=(k == 0), stop=(k == KCH - 1))
                if k == KCH - 1:
                    nc.vector.tensor_scalar_add(
                        qkT_sb[m][:, n * 512:(n + 1) * 512],
                        ps[(m, n)][:, :], qkb_sb[:, m:m + 1])
            target = min(len(sched), max(0, (k - 1) * 2))
            while ci < target:
                emit_chunk(*sched[ci])
                ci += 1
        while ci < len(sched):
            emit_chunk(*sched[ci])
            ci += 1

    proj_cm.__exit__(None, None, None)
    sched3 = score_sched(3)
    ci3 = 0

    # ---- phase 3: PV (natural orientation) + normalize + transposes ------
    ctx_cm = tc.tile_pool(name="cx", bufs=4, space="PSUM")
    ctx_pool = ctx_cm.__enter__()

    cnat = [None] * PAIRS          # [128, 8, 2, 64] normalized ctx, natural
    tp_done = [0] * PAIRS          # transposes emitted per pair (in qb units)
    tp_tiles = {}                  # (p, half) -> psum tile [128, 512] F32

    def emit_tp(p, half, on_dve=False):
        """Transpose 4 qb blocks of pair p's normalized ctx into ctx^T and
        copy to SBUF."""
        tpt = sT_pool.tile([128, 512], BF16, name=f"tp{p}_{half}", tag="sT")
        for qi in range(4):
            qb = half * 4 + qi
            nc.tensor.transpose(tpt[:, qi * 128:(qi + 1) * 128],
                                cnat[p][:, qb, :, :], eye_sb[:, :])
        dst = ctxT_sb[p][:, half * 512:(half + 1) * 512]
        if on_dve:
            nc.vector.tensor_copy(dst, tpt[:, :])
        else:
            nc.scalar.copy(dst, tpt[:, :])
        tp_tiles[(p, half)] = tpt

    for p in range(PAIRS):
        cnat[p] = cnat_pool.tile([128, 8, 2, 64], BF16, name=f"cn{p}",
                                 tag="cn")
        cx = {(h, half): ctx_pool.tile([128, 4, 65], F32,
                                       name=f"cx{p}_{h}_{half}", tag="cx")
              for h in range(2) for half in range(2)}
        for qb in range(8):
            half, qi = qb // 4, qb % 4
            for kb in range(qb + 1):
                if qb < 4:
                    key = (p, kb, 0)
                else:
                    key = (p, kb, 1 if kb < 4 else 0)
                pt, c0, _ = pt_t[key]
                off = qb * 128 - c0
                for h in range(2):
                    nc.tensor.matmul(
                        cx[(h, half)][:, qi, 0:65],
                        lhsT=pt[:, h, off:off + 128],
                        rhs=v_sb[kb][:, (2 * p + h) * 65:(2 * p + h + 1) * 65],
                        start=(kb == 0), stop=(kb == qb))
            if qi == 3:
                # whole half done (diag of its last qb): normalize 4 qb
                # blocks per head in two DVE ops (recip + broadcast mul)
                for h in range(2):
                    rec4 = rec_pool.tile([128, 4, 1], F32,
                                         name=f"rc{p}{half}{h}", tag="rc")
                    nc.vector.reciprocal(rec4[:, :, :],
                                         cx[(h, half)][:, :, 64:65])
                    cslice = cnat[p][:, half * 4:half * 4 + 4, h, :]
                    rec_b, _ = bass.broadcast_tensor_aps(rec4, cslice)
                    nc.vector.tensor_mul(cslice, cx[(h, half)][:, :, 0:64],
                                         rec_b)
            # weave previous pair's transposes into this pair's PV stream
            if p == 0:
                target = min(len(sched3), (qb + 1) * 2)
                while ci3 < target:
                    emit_chunk(*sched3[ci3])
                    ci3 += 1
            if p >= 1 and qb == 0 and tp_done[p - 1] == 0:
                emit_tp(p - 1, 0)
                tp_done[p - 1] = 4
            if p >= 1 and qb == 3 and tp_done[p - 1] == 4:
                emit_tp(p - 1, 1)
                tp_done[p - 1] = 8

    emit_tp(3, 0)
    # bridge the ctx-release chain (pair-3 half-1 norms on DVE) with warmup
    # matmuls so the PE p-state never resets, then transpose pair-3's second
    # half as soon as its norms land
    dmE = sT_pool.tile([128, 2, 512], F32, name="dmE", tag="sT")
    for _ in range(5):
        nc.tensor.matmul(dmE[:, 0, 0:512], lhsT=dmsrc[:, 0:128],
                         rhs=dmsrc[:, 0:512], start=True, stop=True)
    emit_tp(3, 1)

    ctx_cm.__exit__(None, None, None)

    # ---- phase 4: out^T partial = wout.T @ ctx^T, 4 waves of 4 -----------
    ops_cm = tc.tile_pool(name="ops", bufs=4, space="PSUM")
    ops_pool = ops_cm.__enter__()

    dn = [(d, n) for d in range(8) for n in range(2)]
    waves = [dn[i:i + 4] for i in range(0, 16, 4)]
    for wi, wave in enumerate(waves):
        last = wi == len(waves) - 1
        ps = {}
        for (d, n) in wave:
            ps[(d, n)] = ops_pool.tile([128, 512], F32, name=f"o{d}_{n}",
                                       tag="op")
            last_p = 2 if wi == 0 else 3
            for p in range(last_p + 1):
                nc.tensor.matmul(
                    ps[(d, n)][:, :],
                    lhsT=wout_sb[p][:, d * 128:(d + 1) * 128],
                    rhs=ctxT_sb[p][:, n * 512:(n + 1) * 512],
                    start=(p == 0), stop=(p == 3))
        if wi == 0:
            for (d, n) in wave:
                nc.tensor.matmul(
                    ps[(d, n)][:, :],
                    lhsT=wout_sb[3][:, d * 128:(d + 1) * 128],
                    rhs=ctxT_sb[3][:, n * 512:(n + 1) * 512],
                    start=False, stop=True)
        if True:
            for i, (d, n) in enumerate(wave):
                osb = osb_pool.tile([128, 512], BF16, name=f"ob{d}_{n}",
                                    tag="osb")
                on_act = (i % 2 == 0) if wi else (i >= 2)
                if on_act:
                    nc.scalar.activation(osb[:, :], ps[(d, n)][:, :], Ident,
                                         bias=outb_sb[:, d:d + 1])
                else:
                    nc.vector.tensor_scalar_add(osb[:, :], ps[(d, n)][:, :],
                                                outb_sb[:, d:d + 1])
                nc.sync.dma_start(
                    out=outT[d * 128:(d + 1) * 128, n * 512:(n + 1) * 512],
                    in_=osb[:, :])

    ops_cm.__exit__(None, None, None)


_PT_BUFS = {512: 24, 384: 8, 256: 8, 128: 8}


def _build():
    nc = bass.Bass("TRN2", target_bir_lowering=False, debug=False,
                   num_devices=NCORES)
    io = {
        "hsT": nc.dram_tensor("hsT", [1024, S], BF16,
                              kind="ExternalInput").ap(),
        "wqk": nc.dram_tensor("wqk", [1024, 1024], BF16,
                              kind="ExternalInput").ap(),
        "qkb": nc.dram_tensor("qkb", [128, 8], F32,
                              kind="ExternalInput").ap(),
        "wv": nc.dram_tensor("wv", [1024, 512], BF16,
                             kind="ExternalInput").ap(),
        "wout": nc.dram_tensor("wout", [512, 1024], BF16,
                               kind="ExternalInput").ap(),
        "outb": nc.dram_tensor("outb", [128, 8], F32,
                               kind="ExternalInput").ap(),
        "tri": nc.dram_tensor("tri", [128, 128], BF16,
                              kind="ExternalInput").ap(),
        "eye": nc.dram_tensor("eye", [128, 128], BF16,
                              kind="ExternalInput").ap(),
        "outbr": nc.dram_tensor("outbr", [1, 1024], BF16,
                                kind="ExternalInput").ap(),
        "outT": nc.dram_tensor("outT", [1024, S], BF16,
                               kind="ExternalOutput").ap(),
    }
    with tile.TileContext(nc) as tc:
        with ExitStack() as ctx:
            _emit(tc, io, ctx)
    fixed = _legalize_waits_json(nc.to_json_bytes())
    nc.to_json_bytes = (lambda fixed=fixed: fixed)
    return nc


def _get_nc():
    if "nc" not in _CACHE:
        _CACHE["nc"] = _build()
    return _CACHE["nc"]


def _prep_inputs(hidden_states, att_w, att_b, out_w, out_b):
    """Build the 8 per-core input maps (host-side shard/layout prep)."""
    hs = np.asarray(hidden_states, dtype=np.float32)
    att_w = np.asarray(att_w, dtype=np.float32)
    att_b = np.asarray(att_b, dtype=np.float32)
    out_w = np.asarray(out_w, dtype=np.float32)
    out_b = np.asarray(out_b, dtype=np.float32)

    tri = np.triu(np.ones((128, 128), dtype=np.float32)).astype(NPBF16)
    eye = np.eye(128, dtype=np.float32).astype(NPBF16)

    # per-batch / per-head-group pieces are shared between cores
    hsT_all = [np.ascontiguousarray(hs[b].T.astype(NPBF16))
               for b in range(B)]
    per_hg = []
    for hg in range(2):
        lo, hi = hg * 512, (hg + 1) * 512
        wqk = np.ascontiguousarray(
            np.concatenate([att_w[:, lo:hi], att_w[:, D + lo:D + hi]],
                           axis=1).astype(NPBF16))
        qkb = np.concatenate([att_b[lo:hi], att_b[D + lo:D + hi]])
        qkb = np.ascontiguousarray(qkb.reshape(8, 128).T).astype(np.float32)
        wv = np.ascontiguousarray(
            att_w[:, 2 * D + lo:2 * D + hi].astype(NPBF16))
        wout = np.ascontiguousarray(out_w[lo:hi, :].astype(NPBF16))
        # v-bias passes through softmax as a constant (weights sum to 1):
        # ctx = ctx0 + bv, so fold bv @ w_out into this core's output bias.
        corr = att_b[2 * D + lo:2 * D + hi] @ out_w[lo:hi, :]
        outb_eff = (out_b if hg == 0 else 0.0) + corr
        outb_t = np.ascontiguousarray(
            outb_eff.reshape(8, 128).T).astype(np.float32)
        outbr = np.ascontiguousarray(outb_eff.reshape(1, 1024)).astype(NPBF16)
        per_hg.append((wqk, qkb, wv, wout, outb_t, outbr))
    in_maps = []
    for c in range(NCORES):
        b, hg = divmod(c, 2)
        wqk, qkb, wv, wout, outb_t, outbr = per_hg[hg]
        in_maps.append({
            "hsT": hsT_all[b],
            "wqk": wqk,
            "qkb": qkb,
            "wv": wv,
            "wout": wout,
            "outb": outb_t,
            "tri": tri,
            "eye": eye,
            "outbr": outbr,
        })
    return in_maps


def kernel(hidden_states, att_w, att_b, out_w, out_b):
    global LAST_RESULTS
    in_maps = _prep_inputs(hidden_states, att_w, att_b, out_w, out_b)
    nc = _get_nc()
    trace = TRACE
    if trace:
        try:
            from antenv.axon_hooks import get_axon_ntff_profile_hook  # noqa
        except ImportError:
            trace = False
    res = run_bass_kernel_spmd(nc, in_maps, core_ids=list(range(NCORES)),
                               trace=trace)
    LAST_RESULTS = res
    out = np.empty((B, S, D), dtype=np.float32)
    for b in range(B):
        acc = (res.results[2 * b]["outT"].astype(np.float32)
               + res.results[2 * b + 1]["outT"].astype(np.float32))
        out[b] = acc.T
    return out


# revision 29
# speedup vs baseline: 1.0616x; 1.0616x over previous
"""Bark-style causal self-attention on 8 Trainium2 NeuronCores.

Problem (hardcoded): B=4, S=1024, D=1024, H=16, hd=64, fp32 I/O.

Sharding: 8 cores = 4 batches x 2 head-groups (8 heads each).

Single fully-woven pipeline per core, designed to keep the PE tensor engine
continuously busy (the cost model's p-state ramp halves the PE clock after
any idle gap):

  - qk^T projection in 4 k-major waves of 4 PSUM tiles; score matmuls for
    pair w-1 are woven between wave w's projection matmuls, so the Act
    engine's exp marathon (the only engine that can run Exp) hides under
    projection compute.
  - scores are computed transposed per head pair (tile_position row
    packing), exp'd on Act (both heads per instruction), causal-masked on
    DVE.
  - PV runs in NATURAL orientation: stationary = p^T 128x128 blocks,
    moving = V with an appended ones column (65 cols) -> ctx[q, d] comes
    out with the softmax denominator as a per-partition scalar column.
    This halves PV PE-cycles vs streaming queries and makes normalization
    a cheap reciprocal + tensor_scalar_mul per 128-query block.
  - normalized ctx blocks are transposed back with PE identity-matmuls
    (1 cycle/row) into ctx^T for the output projection.
  - out^T partial = w_out.T @ ctx^T accumulated pair 0..2 then pair 3
    (woven with pair 3's transposes) in 4 waves of 4 PSUM tiles; the two
    cores of a batch hold partial sums which are combined on the host.
"""

from contextlib import ExitStack

import numpy as np
import ml_dtypes

import concourse.bass as bass
import concourse.tile as tile
import concourse.mybir as mybir
from concourse.bass_utils import run_bass_kernel_spmd
from concourse.vector_clock import ScopedClock


# --------------------------------------------------------------------------
# Workaround for the walrus build in this container, which accepts at most
# ONE sync-wait command per instruction (two on EventSemaphore).  Stock Tile
# emits instructions with several waits; we legalize the program after
# TileContext exit.
# --------------------------------------------------------------------------

def _patched_drain_and_barrier(self, tick_clock, wait_clock):
    drain_inst = self.nc.sync.drain()
    wait_clock.add_sem_waits(
        drain_inst.ins, ScopedClock({None: tick_clock.global_clock})
    )
    si = drain_inst.ins.sync_info
    waits = list(si.on_wait or []) if si is not None else []
    if len(waits) > 1:
        si.on_wait = [waits[0]]
        for w in waits[1:]:
            extra = self.nc.sync.drain()
            esi = extra.ins.sync_info
            if esi is None:
                extra.ins.sync_info = mybir.SyncInfo(on_wait=[w], on_update=[])
            else:
                esi.on_wait = [w]

    self.nc.all_engine_barrier()
    assert self.sems is not None
    popped = self.nc._tile_sem_poison_stack.pop()
    assert popped is self._sem_poison
    self.nc.clear_and_free_semaphores(list(self.sems.allocated().values()))
    self.nc.all_engine_barrier()


tile.TileContext._drain_and_barrier = _patched_drain_and_barrier


def _legalize_waits_json(raw: bytes) -> bytes:
    """Split multi-wait instructions by inserting single-wait NoOp carriers
    immediately before them on the same engine (pure in-stream split: all
    waits still execute before the instruction, in the same order)."""
    import orjson

    j = orjson.loads(raw)
    for f in j["functions"]:
        for b in f["blocks"]:
            out = []
            for inst in b["instructions"]:
                si = inst.get("sync_info") or {}
                waits = si.get("on_wait") or []
                cap = 2 if inst.get("opcode") == "EventSemaphore" else 1
                if len(waits) > cap:
                    excess, keep = waits[:-cap], waits[-cap:]
                    for k, w in enumerate(excess):
                        out.append({
                            "debug": inst.get("debug", 0),
                            "engine": inst["engine"],
                            "ins": [],
                            "name": f"{inst['name']}-lw{k}",
                            "opcode": "NoOp",
                            "outs": [],
                            "sync_info": {"on_wait": [w]},
                        })
                    si["on_wait"] = keep
                    inst["sync_info"] = si
                out.append(inst)
            b["instructions"] = out
    return orjson.dumps(j)


BF16 = mybir.dt.bfloat16
F32 = mybir.dt.float32
NPBF16 = ml_dtypes.bfloat16

B, S, D, H, HD = 4, 1024, 1024, 16, 64
NCORES = 8
HPC = 8          # heads per core
PAIRS = 4        # head pairs per core
KCH = 8          # 128-row chunks of the D contraction
SCALE = 1.0 / np.sqrt(HD)

# Set by test harness to capture a profile; read back from LAST_RESULTS.
TRACE = False
LAST_RESULTS = None

_CACHE = {}


def _chunks(kb):
    """Column chunks for key-block kb: causal cols [kb*128, S) split at the
    absolute 512 boundary (PSUM bank / q-half boundary)."""
    lo = kb * 128
    if lo < 512:
        return [(lo, 512), (512, 1024)]
    return [(lo, 1024)]


def _emit(tc, io, ctx):
    nc = tc.nc
    hsT, wqk, qkb, wv, wout, outb, tri, eye, outT = (
        io["hsT"], io["wqk"], io["qkb"], io["wv"], io["wout"], io["outb"],
        io["tri"], io["eye"], io["outT"],
    )
    Exp = mybir.ActivationFunctionType.Exp
    Ident = mybir.ActivationFunctionType.Identity

    persist = ctx.enter_context(tc.tile_pool(name="persist", bufs=1))

    def load(name, src, shape, dtype=BF16):
        t = persist.tile(shape, dtype, name=name, tag=name)
        nc.sync.dma_start(out=t[:, :], in_=src)
        return t

    # Warmup source for dummy matmuls (Pool memset, no input deps, runs at
    # t~0).  The dummies keep the PE p-state ramp alive through the
    # load-supply-bound first wave: any PE idle gap halves the modeled PE
    # clock for the next 3us.
    dmsrc = persist.tile([128, 512], BF16, name="dmsrc", tag="dmsrc")
    nc.gpsimd.memset(dmsrc[:, 0:128], 0.0)
    nc.gpsimd.memset(dmsrc[:, 128:512], 0.0)
    dmrd = persist.tile([128, 2], F32, name="dmrd", tag="dmrd")

    # ---- resident SBUF tensors (loads in consumption order) --------------
    # Full-chunk loads: the 625ns HWDGE generation cost per DMA means small
    # slices make supply HWDGE-bound; [128,1024]bf16 keeps it transfer-bound.
    wqk_sb, hsT_sb = [], []
    for k in range(KCH):
        r0, r1 = k * 128, (k + 1) * 128
        wt = persist.tile([128, 1024], BF16, name=f"wqk{k}", tag=f"wqk{k}")
        ht = persist.tile([128, S], BF16, name=f"hsT{k}", tag=f"hsT{k}")
        if k == 0:
            nc.sync.dma_start(out=ht[:, 0:512], in_=hsT[r0:r1, 0:512])
            nc.sync.dma_start(out=wt[:, :], in_=wqk[r0:r1, :])
            nc.sync.dma_start(out=ht[:, 512:1024], in_=hsT[r0:r1, 512:1024])
        else:
            nc.sync.dma_start(out=wt[:, :], in_=wqk[r0:r1, :])
            nc.sync.dma_start(out=ht[:, :], in_=hsT[r0:r1, :])
        wqk_sb.append(wt)
        hsT_sb.append(ht)
    qkb_sb = load("qkb", qkb[:, :], [128, 8], F32)
    wv_sb = [load(f"wv{k}", wv[k * 128:(k + 1) * 128, :], [128, 512])
             for k in range(KCH)]
    wout_sb = [load(f"wout{p}", wout[p * 128:(p + 1) * 128, :], [128, 1024])
               for p in range(PAIRS)]
    outb_sb = load("outb", outb[:, :], [128, 8], F32)
    tri_sb = load("tri", tri[:, :], [128, 128])
    eye_sb = load("eye", eye[:, :], [128, 128])

    # projection outputs
    qkT_sb = [persist.tile([128, S], BF16, name=f"qkT{m}", tag=f"qkT{m}")
              for m in range(8)]   # 0-3: q pairs, 4-7: k pairs
    v_sb = [persist.tile([128, HPC * 65], BF16, name=f"v{s}", tag=f"v{s}")
            for s in range(8)]
    ctxT_sb = [persist.tile([128, S], BF16, name=f"ctxT{p}", tag=f"ctxT{p}")
               for p in range(PAIRS)]

    # rotating SBUF pools
    pt_pool = ctx.enter_context(tc.tile_pool(name="pt", bufs=1))
    cnat_pool = ctx.enter_context(tc.tile_pool(name="cnat", bufs=2))
    rec_pool = ctx.enter_context(tc.tile_pool(name="rec", bufs=8))
    osb_pool = ctx.enter_context(tc.tile_pool(name="osb", bufs=8))
    sums_pool = ctx.enter_context(tc.tile_pool(name="sums", bufs=4))

    # sT psum pool opened first (outlives the sequential proj/ctx/ops pools)
    sT_pool = ctx.enter_context(tc.tile_pool(name="sTp", bufs=2,
                                             space="PSUM"))

    pt_t = {}    # (p, kb, ci) -> (tile, c0, width)

    def emit_chunk(p, kb, ci, c0, c1):
        """Score matmuls (PE) + exp (Act) + causal mask (DVE) for chunk
        (kb, ci) of pair p, covering absolute cols [c0, c1)."""
        wc = c1 - c0
        sT = sT_pool.tile([128, 2, 512], F32, name=f"sT{p}_{kb}_{ci}",
                          tag="sT")
        for t in range(2):
            nc.tensor.matmul(
                sT[:, t, 0:wc],
                lhsT=qkT_sb[4 + p][64 * t:64 * t + 64,
                                   kb * 128:(kb + 1) * 128],
                rhs=qkT_sb[p][64 * t:64 * t + 64, c0:c1],
                start=True, stop=True,
                tile_position=(64 * t, 0))
        pt = pt_pool.tile([128, 2, wc], BF16, name=f"pt{p}_{kb}_{ci}",
                          tag=f"pt{wc}", bufs=_PT_BUFS[wc])
        nc.scalar.activation(pt[:, :, 0:wc], sT[:, :, 0:wc], Exp, scale=SCALE)
        if c0 == kb * 128:
            pm = pt[:, :, 0:128]
            tri3 = tri_sb.rearrange("p (o c) -> p o c", o=1)
            tri_b, _ = bass.broadcast_tensor_aps(tri3, pm)
            nc.vector.tensor_mul(pm, pm, tri_b)
        pt_t[(p, kb, ci)] = (pt, c0, wc)

    def score_sched(p):
        """List of chunk-emit thunks for pair p (12 chunks, kb-major)."""
        out = []
        for kb in range(KCH):
            for ci, (c0, c1) in enumerate(_chunks(kb)):
                out.append((p, kb, ci, c0, c1))
        return out

    # ---- phase 1: qk^T projection, 4 k-major waves + woven scores --------
    proj_cm = tc.tile_pool(name="pj", bufs=4, space="PSUM")
    proj_pool = proj_cm.__enter__()

    # dummy warmup tiles live in the (otherwise still idle) sT slots
    dm = [sT_pool.tile([128, 2, 512], F32, name=f"dm{i}", tag="sT")
          for i in range(2)]

    def dummy_mm(i, cols=512):
        nc.tensor.matmul(dm[i][:, 0, 0:cols], lhsT=dmsrc[:, 0:128],
                         rhs=dmsrc[:, 0:cols], start=True, stop=True)

    for _ in range(3):
        dummy_mm(0, cols=128)
    for _ in range(5):
        dummy_mm(0)

    for w in range(4):
        tiles = [(m, n) for m in (w, 4 + w) for n in range(2)]
        ps = {}
        sched = score_sched(w - 1) if w >= 1 else []
        ci = 0
        for k in range(KCH):
            for (m, n) in tiles:
                if k == 0:
                    ps[(m, n)] = proj_pool.tile([128, 512], F32,
                                                name=f"pj{w}_{m}_{n}",
                                                tag="pj")
                nc.tensor.matmul(
                    ps[(m, n)][:, :],
                    lhsT=wqk_sb[k][:, m * 128:(m + 1) * 128],
                    rhs=hsT_sb[k][:, n * 512:(n + 1) * 512],
                    start=(k == 0), stop=(k == KCH - 1))
                if k == KCH - 1:
                    nc.vector.tensor_scalar_add(
                        qkT_sb[m][:, n * 512:(n + 1) * 512],
                        ps[(m, n)][:, :], qkb_sb[:, m:m + 1])
            if w == 0 and 1 <= k <= 7:
                # wave 0 is load-supply-bound: pad PE with warmup matmuls
                di = 0 if k <= 3 else 1
                for _ in range(3 + (1 if k >= 5 else 0)):
                    dummy_mm(di)
                if k == 3:
                    nc.vector.tensor_copy(dmrd[:, 0:1], dm[0][:, 0, 0:1])
                if k == 7:
                    nc.vector.tensor_copy(dmrd[:, 1:2], dm[1][:, 0, 0:1])
            target = min(len(sched), max(0, (k - 1) * 2))
            while ci < target:
                emit_chunk(*sched[ci])
                ci += 1
        while ci < len(sched):
            emit_chunk(*sched[ci])
            ci += 1

    # ---- phase 2: V projection, s-major sweeps + scores for pair 3 -------
    sched3 = score_sched(3)
    ci3 = 0
    for si in range(8):
        vps = proj_pool.tile([128, 512], F32, name=f"vps{si}", tag="pj")
        for k in range(KCH):
            nc.tensor.matmul(
                vps[:, :],
                lhsT=hsT_sb[k][:, si * 128:(si + 1) * 128],
                rhs=wv_sb[k][:, :],
                start=(k == 0), stop=(k == KCH - 1))
        v3 = v_sb[si].rearrange("p (h c) -> p h c", c=65)
        nc.vector.tensor_copy(v3[:, :, 0:64],
                              vps.rearrange("p (h c) -> p h c", c=64))
        nc.gpsimd.memset(v3[:, :, 64:65], 1.0)
        target = min(len(sched3), max(0, ((si - 1) * 3) // 2))
        while ci3 < target:
            emit_chunk(*sched3[ci3])
            ci3 += 1
    while ci3 < len(sched3):
        emit_chunk(*sched3[ci3])
        ci3 += 1

    proj_cm.__exit__(None, None, None)

    # ---- phase 3: PV (natural orientation) + normalize + transposes ------
    ctx_cm = tc.tile_pool(name="cx", bufs=4, space="PSUM")
    ctx_pool = ctx_cm.__enter__()

    cnat = [None] * PAIRS          # [128, 8, 2, 64] normalized ctx, natural
    tp_done = [0] * PAIRS          # transposes emitted per pair (in qb units)
    tp_tiles = {}                  # (p, half) -> psum tile [128, 512] F32

    def emit_tp(p, half, on_dve=False):
        """Transpose 4 qb blocks of pair p's normalized ctx into ctx^T and
        copy to SBUF."""
        tpt = sT_pool.tile([128, 512], BF16, name=f"tp{p}_{half}", tag="sT")
        for qi in range(4):
            qb = half * 4 + qi
            nc.tensor.transpose(tpt[:, qi * 128:(qi + 1) * 128],
                                cnat[p][:, qb, :, :], eye_sb[:, :])
        dst = ctxT_sb[p][:, half * 512:(half + 1) * 512]
        if on_dve:
            nc.vector.tensor_copy(dst, tpt[:, :])
        else:
            nc.scalar.copy(dst, tpt[:, :])
        tp_tiles[(p, half)] = tpt

    for p in range(PAIRS):
        cnat[p] = cnat_pool.tile([128, 8, 2, 64], BF16, name=f"cn{p}",
                                 tag="cn")
        cx = {(h, half): ctx_pool.tile([128, 4, 65], F32,
                                       name=f"cx{p}_{h}_{half}", tag="cx")
              for h in range(2) for half in range(2)}
        for qb in range(8):
            half, qi = qb // 4, qb % 4
            for kb in range(qb + 1):
                if qb < 4:
                    key = (p, kb, 0)
                else:
                    key = (p, kb, 1 if kb < 4 else 0)
                pt, c0, _ = pt_t[key]
                off = qb * 128 - c0
                for h in range(2):
                    nc.tensor.matmul(
                        cx[(h, half)][:, qi, 0:65],
                        lhsT=pt[:, h, off:off + 128],
                        rhs=v_sb[kb][:, (2 * p + h) * 65:(2 * p + h + 1) * 65],
                        start=(kb == 0), stop=(kb == qb))
            if qi == 3:
                # whole half done (diag of its last qb): normalize 4 qb
                # blocks per head in two DVE ops (recip + broadcast mul)
                for h in range(2):
                    rec4 = rec_pool.tile([128, 4, 1], F32,
                                         name=f"rc{p}{half}{h}", tag="rc")
                    nc.vector.reciprocal(rec4[:, :, :],
                                         cx[(h, half)][:, :, 64:65])
                    cslice = cnat[p][:, half * 4:half * 4 + 4, h, :]
                    rec_b, _ = bass.broadcast_tensor_aps(rec4, cslice)
                    nc.vector.tensor_mul(cslice, cx[(h, half)][:, :, 0:64],
                                         rec_b)
            # weave previous pair's transposes into this pair's PV stream
            if p >= 1 and qb == 0 and tp_done[p - 1] == 0:
                emit_tp(p - 1, 0)
                tp_done[p - 1] = 4
            if p >= 1 and qb == 3 and tp_done[p - 1] == 4:
                emit_tp(p - 1, 1)
                tp_done[p - 1] = 8

    emit_tp(3, 0)
    # bridge the ctx-release chain (pair-3 half-1 norms on DVE) with warmup
    # matmuls so the PE p-state never resets, then transpose pair-3's second
    # half as soon as its norms land
    dmE = sT_pool.tile([128, 2, 512], F32, name="dmE", tag="sT")
    for _ in range(5):
        nc.tensor.matmul(dmE[:, 0, 0:512], lhsT=dmsrc[:, 0:128],
                         rhs=dmsrc[:, 0:512], start=True, stop=True)
    emit_tp(3, 1)

    ctx_cm.__exit__(None, None, None)

    # ---- phase 4: out^T partial = wout.T @ ctx^T, 4 waves of 4 -----------
    ops_cm = tc.tile_pool(name="ops", bufs=4, space="PSUM")
    ops_pool = ops_cm.__enter__()

    dn = [(d, n) for d in range(8) for n in range(2)]
    waves = [dn[i:i + 4] for i in range(0, 16, 4)]
    for wi, wave in enumerate(waves):
        last = wi == len(waves) - 1
        ps = {}
        for (d, n) in wave:
            ps[(d, n)] = ops_pool.tile([128, 512], F32, name=f"o{d}_{n}",
                                       tag="op")
            last_p = 2 if wi == 0 else 3
            for p in range(last_p + 1):
                nc.tensor.matmul(
                    ps[(d, n)][:, :],
                    lhsT=wout_sb[p][:, d * 128:(d + 1) * 128],
                    rhs=ctxT_sb[p][:, n * 512:(n + 1) * 512],
                    start=(p == 0), stop=(p == 3))
        if wi == 0:
            for (d, n) in wave:
                nc.tensor.matmul(
                    ps[(d, n)][:, :],
                    lhsT=wout_sb[3][:, d * 128:(d + 1) * 128],
                    rhs=ctxT_sb[3][:, n * 512:(n + 1) * 512],
                    start=False, stop=True)
        if True:
            for i, (d, n) in enumerate(wave):
                osb = osb_pool.tile([128, 512], BF16, name=f"ob{d}_{n}",
                                    tag="osb")
                on_act = (i % 2 == 0) if wi else (i >= 2)
                if on_act:
                    nc.scalar.activation(osb[:, :], ps[(d, n)][:, :], Ident,
                                         bias=outb_sb[:, d:d + 1])
                else:
                    nc.vector.tensor_scalar_add(osb[:, :], ps[(d, n)][:, :],
                                                outb_sb[:, d:d + 1])
                nc.sync.dma_start(
                    out=outT[d * 128:(d + 1) * 128, n * 512:(n + 1) * 512],
                    in_=osb[:, :])

    ops_cm.__exit__(None, None, None)


_PT_BUFS = {512: 24, 384: 8, 256: 8, 128: 8}


def _build():
    nc = bass.Bass("TRN2", target_bir_lowering=False, debug=False,
                   num_devices=NCORES)
    io = {
        "hsT": nc.dram_tensor("hsT", [1024, S], BF16,
                              kind="ExternalInput").ap(),
        "wqk": nc.dram_tensor("wqk", [1024, 1024], BF16,
                              kind="ExternalInput").ap(),
        "qkb": nc.dram_tensor("qkb", [128, 8], F32,
                              kind="ExternalInput").ap(),
        "wv": nc.dram_tensor("wv", [1024, 512], BF16,
                             kind="ExternalInput").ap(),
        "wout": nc.dram_tensor("wout", [512, 1024], BF16,
                               kind="ExternalInput").ap(),
        "outb": nc.dram_tensor("outb", [128, 8], F32,
                               kind="ExternalInput").ap(),
        "tri": nc.dram_tensor("tri", [128, 128], BF16,
                              kind="ExternalInput").ap(),
        "eye": nc.dram_tensor("eye", [128, 128], BF16,
                              kind="ExternalInput").ap(),
        "outbr": nc.dram_tensor("outbr", [1, 1024], BF16,
                                kind="ExternalInput").ap(),
        "outT": nc.dram_tensor("outT", [1024, S], BF16,
                               kind="ExternalOutput").ap(),
    }
    with tile.TileContext(nc) as tc:
        with ExitStack() as ctx:
            _emit(tc, io, ctx)
    fixed = _legalize_waits_json(nc.to_json_bytes())
    nc.to_json_bytes = (lambda fixed=fixed: fixed)
    return nc


def _get_nc():
    if "nc" not in _CACHE:
        _CACHE["nc"] = _build()
    return _CACHE["nc"]


def _prep_inputs(hidden_states, att_w, att_b, out_w, out_b):
    """Build the 8 per-core input maps (host-side shard/layout prep)."""
    hs = np.asarray(hidden_states, dtype=np.float32)
    att_w = np.asarray(att_w, dtype=np.float32)
    att_b = np.asarray(att_b, dtype=np.float32)
    out_w = np.asarray(out_w, dtype=np.float32)
    out_b = np.asarray(out_b, dtype=np.float32)

    tri = np.triu(np.ones((128, 128), dtype=np.float32)).astype(NPBF16)
    eye = np.eye(128, dtype=np.float32).astype(NPBF16)

    # per-batch / per-head-group pieces are shared between cores
    hsT_all = [np.ascontiguousarray(hs[b].T.astype(NPBF16))
               for b in range(B)]
    per_hg = []
    for hg in range(2):
        lo, hi = hg * 512, (hg + 1) * 512
        wqk = np.ascontiguousarray(
            np.concatenate([att_w[:, lo:hi], att_w[:, D + lo:D + hi]],
                           axis=1).astype(NPBF16))
        qkb = np.concatenate([att_b[lo:hi], att_b[D + lo:D + hi]])
        qkb = np.ascontiguousarray(qkb.reshape(8, 128).T).astype(np.float32)
        wv = np.ascontiguousarray(
            att_w[:, 2 * D + lo:2 * D + hi].astype(NPBF16))
        wout = np.ascontiguousarray(out_w[lo:hi, :].astype(NPBF16))
        # v-bias passes through softmax as a constant (weights sum to 1):
        # ctx = ctx0 + bv, so fold bv @ w_out into this core's output bias.
        corr = att_b[2 * D + lo:2 * D + hi] @ out_w[lo:hi, :]
        outb_eff = (out_b if hg == 0 else 0.0) + corr
        outb_t = np.ascontiguousarray(
            outb_eff.reshape(8, 128).T).astype(np.float32)
        outbr = np.ascontiguousarray(outb_eff.reshape(1, 1024)).astype(NPBF16)
        per_hg.append((wqk, qkb, wv, wout, outb_t, outbr))
    in_maps = []
    for c in range(NCORES):
        b, hg = divmod(c, 2)
        wqk, qkb, wv, wout, outb_t, outbr = per_hg[hg]
        in_maps.append({
            "hsT": hsT_all[b],
            "wqk": wqk,
            "qkb": qkb,
            "wv": wv,
            "wout": wout,
            "outb": outb_t,
            "tri": tri,
            "eye": eye,
            "outbr": outbr,
        })
    return in_maps


def kernel(hidden_states, att_w, att_b, out_w, out_b):
    global LAST_RESULTS
    in_maps = _prep_inputs(hidden_states, att_w, att_b, out_w, out_b)
    nc = _get_nc()
    trace = TRACE
    if trace:
        try:
            from antenv.axon_hooks import get_axon_ntff_profile_hook  # noqa
        except ImportError:
            trace = False
    res = run_bass_kernel_spmd(nc, in_maps, core_ids=list(range(NCORES)),
                               trace=trace)
    LAST_RESULTS = res
    out = np.empty((B, S, D), dtype=np.float32)
    for b in range(B):
        acc = (res.results[2 * b]["outT"].astype(np.float32)
               + res.results[2 * b + 1]["outT"].astype(np.float32))
        out[b] = acc.T
    return out
